# revision 3
# baseline (speedup 1.0000x reference)
"""DCNv2 deformable PS-RoI pooling on 8 Trainium2 NeuronCores.

Strategy (RoI-data-parallel, 32 rois per core):
  * Host replicates the reference coordinate math exactly in float32 and folds
    bilinear weights, validity masking and the 1/count normalization into a
    per-roi sparse matrix A (bbox_pixels x 49). Each roi touches only a small
    bbox of the 64x64 feature map, so A has ~128-384 rows (padded to 128k).
  * Feature map is transposed to channel-last (B*H*W, C) on host so each pixel
    is a contiguous 1KB channel vector in HBM.
  * Device (SPMD, identical program on 8 cores, per-core data in DRAM inputs):
      - one indirect-DMA gather per column group: patch[p, t, :] =
        Fcl[idx[p, t], :]  -> pixel-on-partition layout
      - per roi: out(c,j) accumulated in PSUM over 128-pixel chunks via
        matmul(lhsT=patch_chunk(128px, 128c), rhs=A_chunk(128px, 49j))
      - PSUM -> SBUF staging -> one contiguous DMA to HBM (c-major scratch
        layout); host undoes the layout permutation while assembling.
"""
import numpy as np

f32 = np.float32
f64 = np.float64

B, C, H, W = 8, 256, 64, 64
N_ROIS, P, S = 256, 7, 4
PART = 7
NJ = P * P  # 49
SCALE = f32(1.0 / 16.0)
TRANS_STD = f32(0.1)
N_CORES = 8
RPC = N_ROIS // N_CORES  # rois per core
N_GROUPS = 4  # gather/compute pipeline groups

_prog_cache = {}


# --------------------------------------------------------------------------
# host math: exact f32 replication of the reference coordinate computation
# --------------------------------------------------------------------------
def _roi_sampling_data(rois, offset):
    rois = np.asarray(rois, dtype=f32)
    offset = np.asarray(offset, dtype=f32)
    batch = rois[:, 0].astype(np.int32)

    roi_sw = np.round(rois[:, 1]) * SCALE - f32(0.5)
    roi_sh = np.round(rois[:, 2]) * SCALE - f32(0.5)
    roi_ew = (np.round(rois[:, 3]) + f32(1.0)) * SCALE - f32(0.5)
    roi_eh = (np.round(rois[:, 4]) + f32(1.0)) * SCALE - f32(0.5)
    roi_w = np.maximum(roi_ew - roi_sw, f32(0.1))
    roi_h = np.maximum(roi_eh - roi_sh, f32(0.1))
    bin_w = roi_w / f32(P)
    bin_h = roi_h / f32(P)
    sub_w = bin_w / f32(S)
    sub_h = bin_h / f32(S)

    ph = np.arange(P, dtype=np.int32)
    pw = np.arange(P, dtype=np.int32)
    part_h = np.clip(
        np.floor(ph.astype(f32) / f32(P) * f32(PART)).astype(np.int32), 0, PART - 1
    )
    part_w = np.clip(
        np.floor(pw.astype(f32) / f32(P) * f32(PART)).astype(np.int32), 0, PART - 1
    )

    tx = offset[:, 0][:, part_h[:, None], part_w[None, :]] * TRANS_STD  # (N,7,7)
    ty = offset[:, 1][:, part_h[:, None], part_w[None, :]] * TRANS_STD

    wstart = (
        pw.astype(f32)[None, None, :] * bin_w[:, None, None]
        + roi_sw[:, None, None]
        + tx * roi_w[:, None, None]
    )
    hstart = (
        ph.astype(f32)[None, :, None] * bin_h[:, None, None]
        + roi_sh[:, None, None]
        + ty * roi_h[:, None, None]
    )

    iw = np.arange(S, dtype=f32)
    ih = np.arange(S, dtype=f32)
    wpos = (
        wstart[:, :, :, None, None]
        + iw[None, None, None, None, :] * sub_w[:, None, None, None, None]
    )
    hpos = (
        hstart[:, :, :, None, None]
        + ih[None, None, None, :, None] * sub_h[:, None, None, None, None]
    )

    valid = (
        (wpos >= f32(-0.5)) & (wpos <= f32(W) - f32(0.5))
        & (hpos >= f32(-0.5)) & (hpos <= f32(H) - f32(0.5))
    )
    wc = np.clip(wpos, f32(0.0), f32(W - 1.0))
    hc = np.clip(hpos, f32(0.0), f32(H - 1.0))

    x0 = np.floor(wc).astype(np.int32)
    x1 = np.ceil(wc).astype(np.int32)
    y0 = np.floor(hc).astype(np.int32)
    y1 = np.ceil(hc).astype(np.int32)
    dx = (wc - np.floor(wc)).astype(f64)
    dy = (hc - np.floor(hc)).astype(f64)

    cnt = valid.sum(axis=(3, 4)).astype(f32)  # (N,7,7)
    coef = np.where(cnt > 0, 1.0 / np.maximum(cnt, f32(1.0)).astype(f64), 0.0)

    w00 = (1.0 - dx) * (1.0 - dy)
    w01 = dx * (1.0 - dy)
    w10 = (1.0 - dx) * dy
    w11 = dx * dy

    return dict(
        batch=batch, valid=valid, x0=x0, x1=x1, y0=y0, y1=y1,
        w00=w00, w01=w01, w10=w10, w11=w11, coef=coef,
    )


def _build_roi_mats(rois, offset):
    """Per roi: (pixel idx int32 (npix,), A f32 (npix, 49)), npix % 128 == 0."""
    d = _roi_sampling_data(rois, offset)
    j_grid = np.arange(NJ, dtype=np.int64).reshape(P, P, 1, 1)
    j_grid = np.broadcast_to(j_grid, (P, P, S, S))
    full = (P, P, S, S)

    out = []
    for n in range(N_ROIS):
        v = d["valid"][n]
        if not v.any():
            out.append((np.zeros(128, np.int32), np.zeros((128, NJ), f32)))
            continue
        jj = j_grid[v]
        xs0 = np.broadcast_to(d["x0"][n], full)[v]
        xs1 = np.broadcast_to(d["x1"][n], full)[v]
        ys0 = np.broadcast_to(d["y0"][n], full)[v]
        ys1 = np.broadcast_to(d["y1"][n], full)[v]
        cf = np.broadcast_to(d["coef"][n][:, :, None, None], full)[v]
        bx0 = int(xs0.min()); bx1 = int(xs1.max())
        by0 = int(ys0.min()); by1 = int(ys1.max())
        bw = bx1 - bx0 + 1
        bh = by1 - by0 + 1
        npix = bh * bw
        npad = (-npix) % 128
        A = np.zeros((npix + npad, NJ), f64)
        for yy, xx, ww in (
            (ys0, xs0, np.broadcast_to(d["w00"][n], full)[v]),
            (ys0, xs1, np.broadcast_to(d["w01"][n], full)[v]),
            (ys1, xs0, np.broadcast_to(d["w10"][n], full)[v]),
            (ys1, xs1, np.broadcast_to(d["w11"][n], full)[v]),
        ):
            lp = (yy - by0).astype(np.int64) * bw + (xx - bx0)
            np.add.at(A, (lp, jj), ww * cf)
        yidx = (by0 + np.arange(bh, dtype=np.int32))[:, None]
        xidx = (bx0 + np.arange(bw, dtype=np.int32))[None, :]
        gidx = (int(d["batch"][n]) * (H * W) + yidx * W + xidx).reshape(-1)
        gidx = np.concatenate([gidx, np.zeros(npad, np.int32)]).astype(np.int32)
        out.append((gidx, A.astype(f32)))
    return out


# --------------------------------------------------------------------------
# device program
# --------------------------------------------------------------------------
def _build_program(nch):
    """nch: tuple of RPC ints = chunks per roi slot. Same program on 8 cores."""
    import concourse.bacc as bacc
    import concourse.bass as bass
    import concourse.mybir as mybir
    from concourse.tile import TileContext

    T = int(sum(nch))
    col0 = np.concatenate([[0], np.cumsum(nch)]).astype(int)  # slot -> first col

    # split slots into N_GROUPS groups with roughly equal column counts
    bounds = [0]
    for g in range(1, N_GROUPS):
        target = T * g / N_GROUPS
        s = int(np.searchsorted(col0, target))
        s = min(max(s, bounds[-1] + 1), RPC - (N_GROUPS - g))
        bounds.append(s)
    bounds.append(RPC)

    nc = bacc.Bacc("TRN2", num_devices=N_CORES)
    dt = mybir.dt
    fcl = nc.dram_tensor("fcl", [B * H * W, C], dt.float32, kind="ExternalInput")
    amat = nc.dram_tensor("amat", [128, T, NJ], dt.float32, kind="ExternalInput")
    # dma_gather index layout: logical idx i lives at [i % 16, i // 16],
    # replicated across the 8 groups of 16 partitions.
    pidx = nc.dram_tensor("pidx", [128, T * 8], dt.int16, kind="ExternalInput")
    outd = nc.dram_tensor("out", [128, RPC, 2, NJ], dt.float32, kind="ExternalOutput")

    with TileContext(nc) as tc:
        with (
            tc.tile_pool(name="main", bufs=1) as mp,
            tc.tile_pool(name="psum", bufs=2, space="PSUM") as pp,
        ):
            idx_t = mp.tile([128, T * 8], dt.int16, tag="idx")
            nc.sync.dma_start(out=idx_t[:], in_=pidx[:])

            for g in range(N_GROUPS):
                s0, s1 = bounds[g], bounds[g + 1]
                c0, c1 = int(col0[s0]), int(col0[s1])
                ncols = c1 - c0
                a_t = mp.tile([128, ncols, NJ], dt.float32, tag=f"amat{g}")
                nc.sync.dma_start(out=a_t[:], in_=amat[:, c0:c1, :])
                p_t = mp.tile([128, ncols, C], dt.float32, tag=f"patch{g}")
                nc.gpsimd.dma_gather(
                    out_ap=p_t[:],
                    in_ap=fcl[:],
                    idxs_ap=idx_t[:, c0 * 8:c1 * 8],
                    num_idxs=ncols * 128,
                    num_idxs_reg=ncols * 128,
                    elem_size=C,
                    single_packet=False,
                )
                ob = mp.tile([128, s1 - s0, 2, NJ], dt.float32, tag=f"outbuf{g}")
                for r in range(s0, s1):
                    for h in range(2):
                        ps = pp.tile([128, NJ], dt.float32, tag=f"ps{h}")
                        for t in range(nch[r]):
                            c = int(col0[r]) - c0 + t
                            nc.tensor.matmul(
                                out=ps[:],
                                lhsT=p_t[:, c, h * 128:(h + 1) * 128],
                                rhs=a_t[:, c, :],
                                start=(t == 0),
                                stop=(t == nch[r] - 1),
                            )
                        nc.vector.tensor_copy(out=ob[:, r - s0, h, :], in_=ps[:])
                nc.sync.dma_start(out=outd[:, s0:s1, :, :], in_=ob[:])
    nc.compile()
    return nc


# --------------------------------------------------------------------------
# entry point
# --------------------------------------------------------------------------
def kernel(input, rois, offset):
    from concourse.bass_utils import run_bass_kernel_spmd

    input = np.asarray(input, dtype=f32)
    mats = _build_roi_mats(rois, offset)

    fcl = np.ascontiguousarray(input.transpose(0, 2, 3, 1)).reshape(B * H * W, C)

    # slot assignment: per core, rois sorted by #chunks desc
    slots = []  # slots[k][r] = original roi index
    chunks_per = np.array([len(g) // 128 for g, _ in mats])
    for k in range(N_CORES):
        idxs = np.arange(k * RPC, (k + 1) * RPC)
        order = np.argsort(-chunks_per[idxs], kind="stable")
        slots.append(idxs[order])
    nch = tuple(
        int(max(chunks_per[slots[k][r]] for k in range(N_CORES))) for r in range(RPC)
    )
    T = int(sum(nch))
    col0 = np.concatenate([[0], np.cumsum(nch)]).astype(int)

    key = nch
    if key not in _prog_cache:
        _prog_cache[key] = _build_program(nch)
    nc = _prog_cache[key]

    in_maps = []
    for k in range(N_CORES):
        logical = np.zeros(T * 128, np.int32)
        a_arr = np.zeros((128, T, NJ), f32)
        for r in range(RPC):
            gidx, A = mats[slots[k][r]]
            tchunks = len(gidx) // 128
            for t in range(tchunks):
                col = int(col0[r]) + t
                logical[col * 128:(col + 1) * 128] = gidx[t * 128:(t + 1) * 128]
                a_arr[:, col, :] = A[t * 128:(t + 1) * 128, :]
        # wrap-16 + replicate to 128 partitions (see _build_program)
        idx16 = np.tile(logical.astype(np.int16).reshape(-1, 16).T, (8, 1))
        in_maps.append({"fcl": fcl, "amat": a_arr, "pidx": idx16})

    res = run_bass_kernel_spmd(nc, in_maps, core_ids=list(range(N_CORES)))

    out_full = np.empty((N_ROIS, C, P, P), f32)
    for k in range(N_CORES):
        arr = res.results[k]["out"]  # (128, RPC, 2, 49)
        t = arr.transpose(1, 2, 0, 3).reshape(RPC, C, P, P)
        out_full[slots[k]] = t
    return out_full


# revision 5
# speedup vs baseline: 1.2261x; 1.2261x over previous
"""DCNv2 deformable PS-RoI pooling on 8 Trainium2 NeuronCores.

Strategy (RoI-data-parallel, 32 rois per core):
  * Host replicates the reference coordinate math exactly in float32 and folds
    bilinear weights, validity masking and the 1/count normalization into a
    per-roi sparse matrix A (bbox_pixels x 49). Each roi touches only a small
    bbox of the 64x64 feature map, so A has ~128-384 rows (padded to 128k).
  * Feature map is transposed to channel-last (B*H*W, C) on host so each pixel
    is a contiguous 1KB channel vector in HBM.
  * Device (SPMD, identical program on 8 cores, per-core data in DRAM inputs):
      - one indirect-DMA gather per column group: patch[p, t, :] =
        Fcl[idx[p, t], :]  -> pixel-on-partition layout
      - per roi: out(c,j) accumulated in PSUM over 128-pixel chunks via
        matmul(lhsT=patch_chunk(128px, 128c), rhs=A_chunk(128px, 49j))
      - PSUM -> SBUF staging -> one contiguous DMA to HBM (c-major scratch
        layout); host undoes the layout permutation while assembling.
"""
import numpy as np

f32 = np.float32
f64 = np.float64

B, C, H, W = 8, 256, 64, 64
N_ROIS, P, S = 256, 7, 4
PART = 7
NJ = P * P  # 49
SCALE = f32(1.0 / 16.0)
TRANS_STD = f32(0.1)
N_CORES = 8
RPC = N_ROIS // N_CORES  # rois per core
N_GROUPS = 4  # gather/compute pipeline groups

_prog_cache = {}


# --------------------------------------------------------------------------
# host math: exact f32 replication of the reference coordinate computation
# --------------------------------------------------------------------------
def _roi_sampling_data(rois, offset):
    rois = np.asarray(rois, dtype=f32)
    offset = np.asarray(offset, dtype=f32)
    batch = rois[:, 0].astype(np.int32)

    roi_sw = np.round(rois[:, 1]) * SCALE - f32(0.5)
    roi_sh = np.round(rois[:, 2]) * SCALE - f32(0.5)
    roi_ew = (np.round(rois[:, 3]) + f32(1.0)) * SCALE - f32(0.5)
    roi_eh = (np.round(rois[:, 4]) + f32(1.0)) * SCALE - f32(0.5)
    roi_w = np.maximum(roi_ew - roi_sw, f32(0.1))
    roi_h = np.maximum(roi_eh - roi_sh, f32(0.1))
    bin_w = roi_w / f32(P)
    bin_h = roi_h / f32(P)
    sub_w = bin_w / f32(S)
    sub_h = bin_h / f32(S)

    ph = np.arange(P, dtype=np.int32)
    pw = np.arange(P, dtype=np.int32)
    part_h = np.clip(
        np.floor(ph.astype(f32) / f32(P) * f32(PART)).astype(np.int32), 0, PART - 1
    )
    part_w = np.clip(
        np.floor(pw.astype(f32) / f32(P) * f32(PART)).astype(np.int32), 0, PART - 1
    )

    tx = offset[:, 0][:, part_h[:, None], part_w[None, :]] * TRANS_STD  # (N,7,7)
    ty = offset[:, 1][:, part_h[:, None], part_w[None, :]] * TRANS_STD

    wstart = (
        pw.astype(f32)[None, None, :] * bin_w[:, None, None]
        + roi_sw[:, None, None]
        + tx * roi_w[:, None, None]
    )
    hstart = (
        ph.astype(f32)[None, :, None] * bin_h[:, None, None]
        + roi_sh[:, None, None]
        + ty * roi_h[:, None, None]
    )

    iw = np.arange(S, dtype=f32)
    ih = np.arange(S, dtype=f32)
    wpos = (
        wstart[:, :, :, None, None]
        + iw[None, None, None, None, :] * sub_w[:, None, None, None, None]
    )
    hpos = (
        hstart[:, :, :, None, None]
        + ih[None, None, None, :, None] * sub_h[:, None, None, None, None]
    )

    valid = (
        (wpos >= f32(-0.5)) & (wpos <= f32(W) - f32(0.5))
        & (hpos >= f32(-0.5)) & (hpos <= f32(H) - f32(0.5))
    )
    wc = np.clip(wpos, f32(0.0), f32(W - 1.0))
    hc = np.clip(hpos, f32(0.0), f32(H - 1.0))

    x0 = np.floor(wc).astype(np.int32)
    x1 = np.ceil(wc).astype(np.int32)
    y0 = np.floor(hc).astype(np.int32)
    y1 = np.ceil(hc).astype(np.int32)
    dx = (wc - np.floor(wc)).astype(f64)
    dy = (hc - np.floor(hc)).astype(f64)

    cnt = valid.sum(axis=(3, 4)).astype(f32)  # (N,7,7)
    coef = np.where(cnt > 0, 1.0 / np.maximum(cnt, f32(1.0)).astype(f64), 0.0)

    w00 = (1.0 - dx) * (1.0 - dy)
    w01 = dx * (1.0 - dy)
    w10 = (1.0 - dx) * dy
    w11 = dx * dy

    return dict(
        batch=batch, valid=valid, x0=x0, x1=x1, y0=y0, y1=y1,
        w00=w00, w01=w01, w10=w10, w11=w11, coef=coef,
    )


def _build_roi_mats(rois, offset):
    """Per roi: (pixel idx int32 (npix,), A f32 (npix, 49)), npix % 128 == 0."""
    d = _roi_sampling_data(rois, offset)
    j_grid = np.arange(NJ, dtype=np.int64).reshape(P, P, 1, 1)
    j_grid = np.broadcast_to(j_grid, (P, P, S, S))
    full = (P, P, S, S)

    out = []
    for n in range(N_ROIS):
        v = d["valid"][n]
        if not v.any():
            out.append((np.zeros(128, np.int32), np.zeros((128, NJ), f32)))
            continue
        jj = j_grid[v]
        xs0 = np.broadcast_to(d["x0"][n], full)[v]
        xs1 = np.broadcast_to(d["x1"][n], full)[v]
        ys0 = np.broadcast_to(d["y0"][n], full)[v]
        ys1 = np.broadcast_to(d["y1"][n], full)[v]
        cf = np.broadcast_to(d["coef"][n][:, :, None, None], full)[v]
        bx0 = int(xs0.min()); bx1 = int(xs1.max())
        by0 = int(ys0.min()); by1 = int(ys1.max())
        bw = bx1 - bx0 + 1
        bh = by1 - by0 + 1
        npix = bh * bw
        npad = (-npix) % 128
        A = np.zeros((npix + npad, NJ), f64)
        for yy, xx, ww in (
            (ys0, xs0, np.broadcast_to(d["w00"][n], full)[v]),
            (ys0, xs1, np.broadcast_to(d["w01"][n], full)[v]),
            (ys1, xs0, np.broadcast_to(d["w10"][n], full)[v]),
            (ys1, xs1, np.broadcast_to(d["w11"][n], full)[v]),
        ):
            lp = (yy - by0).astype(np.int64) * bw + (xx - bx0)
            np.add.at(A, (lp, jj), ww * cf)
        yidx = (by0 + np.arange(bh, dtype=np.int32))[:, None]
        xidx = (bx0 + np.arange(bw, dtype=np.int32))[None, :]
        gidx = (int(d["batch"][n]) * (H * W) + yidx * W + xidx).reshape(-1)
        gidx = np.concatenate([gidx, np.zeros(npad, np.int32)]).astype(np.int32)
        out.append((gidx, A.astype(f32)))
    return out


# --------------------------------------------------------------------------
# device program
# --------------------------------------------------------------------------
def _build_program(nch):
    """nch: tuple of RPC ints = chunks per roi slot. Same program on 8 cores."""
    import concourse.bacc as bacc
    import concourse.bass as bass
    import concourse.mybir as mybir
    from concourse.tile import TileContext

    T = int(sum(nch))
    col0 = np.concatenate([[0], np.cumsum(nch)]).astype(int)  # slot -> first col

    # split slots into N_GROUPS groups with roughly equal column counts
    bounds = [0]
    for g in range(1, N_GROUPS):
        target = T * g / N_GROUPS
        s = int(np.searchsorted(col0, target))
        s = min(max(s, bounds[-1] + 1), RPC - (N_GROUPS - g))
        bounds.append(s)
    bounds.append(RPC)

    nc = bacc.Bacc("TRN2", num_devices=N_CORES)
    dt = mybir.dt
    fcl = nc.dram_tensor("fcl", [B * H * W, C], dt.float32, kind="ExternalInput")
    amat = nc.dram_tensor("amat", [128, T, NJ], dt.float32, kind="ExternalInput")
    # dma_gather index layout: logical idx i lives at [i % 16, i // 16],
    # replicated across the 8 groups of 16 partitions.
    pidx = nc.dram_tensor("pidx", [128, T * 8], dt.int16, kind="ExternalInput")
    outd = nc.dram_tensor("out", [128, RPC, 2, NJ], dt.float32, kind="ExternalOutput")

    with TileContext(nc) as tc:
        with (
            tc.tile_pool(name="main", bufs=1) as mp,
            tc.tile_pool(name="psum", bufs=2, space="PSUM") as pp,
        ):
            idx_t = mp.tile([128, T * 8], dt.int16, tag="idx")
            nc.sync.dma_start(out=idx_t[:], in_=pidx[:])

            for g in range(N_GROUPS):
                s0, s1 = bounds[g], bounds[g + 1]
                c0, c1 = int(col0[s0]), int(col0[s1])
                ncols = c1 - c0
                a_t = mp.tile([128, ncols, NJ], dt.float32, tag=f"amat{g}")
                nc.sync.dma_start(out=a_t[:], in_=amat[:, c0:c1, :])
                p_t = mp.tile([128, ncols, C], dt.float32, tag=f"patch{g}")
                nc.gpsimd.dma_gather(
                    out_ap=p_t[:],
                    in_ap=fcl[:],
                    idxs_ap=idx_t[:, c0 * 8:c1 * 8],
                    num_idxs=ncols * 128,
                    num_idxs_reg=ncols * 128,
                    elem_size=C,
                    single_packet=False,
                )
                ob = mp.tile([128, s1 - s0, 2, NJ], dt.float32, tag=f"outbuf{g}")
                # pack 5 rois (10 roi-halves x 49) per PSUM bank; one DVE
                # copy per bank instead of one per roi-half
                for b0 in range(s0, s1, 5):
                    b1 = min(b0 + 5, s1)
                    nsl = (b1 - b0) * 2
                    pb = pp.tile([128, nsl * NJ], dt.float32, tag="pbank")
                    for r in range(b0, b1):
                        for h in range(2):
                            o = ((r - b0) * 2 + h) * NJ
                            for t in range(nch[r]):
                                c = int(col0[r]) - c0 + t
                                nc.tensor.matmul(
                                    out=pb[:, o:o + NJ],
                                    lhsT=p_t[:, c, h * 128:(h + 1) * 128],
                                    rhs=a_t[:, c, :],
                                    start=(t == 0),
                                    stop=(t == nch[r] - 1),
                                )
                    nc.vector.tensor_copy(
                        out=ob[:, b0 - s0:b1 - s0, :, :], in_=pb[:, :nsl * NJ]
                    )
                nc.sync.dma_start(out=outd[:, s0:s1, :, :], in_=ob[:])
    nc.compile()
    return nc


# --------------------------------------------------------------------------
# entry point
# --------------------------------------------------------------------------
def kernel(input, rois, offset):
    from concourse.bass_utils import run_bass_kernel_spmd

    input = np.asarray(input, dtype=f32)
    mats = _build_roi_mats(rois, offset)

    fcl = np.ascontiguousarray(input.transpose(0, 2, 3, 1)).reshape(B * H * W, C)

    # snake-deal rois to cores by descending chunk count so every slot r
    # holds 8 near-equal-size rois -> per-slot max (nch) is tight
    chunks_per = np.array([len(g) // 128 for g, _ in mats])
    order = np.argsort(-chunks_per, kind="stable")
    slots = [[None] * RPC for _ in range(N_CORES)]  # slots[k][r] = roi index
    for i, roi in enumerate(order):
        rnd, pos = divmod(i, N_CORES)
        core = pos if rnd % 2 == 0 else N_CORES - 1 - pos
        slots[core][rnd] = int(roi)
    slots = [np.array(s) for s in slots]
    nch = tuple(
        int(max(chunks_per[slots[k][r]] for k in range(N_CORES))) for r in range(RPC)
    )
    T = int(sum(nch))
    col0 = np.concatenate([[0], np.cumsum(nch)]).astype(int)

    key = nch
    if key not in _prog_cache:
        _prog_cache[key] = _build_program(nch)
    nc = _prog_cache[key]

    in_maps = []
    for k in range(N_CORES):
        logical = np.zeros(T * 128, np.int32)
        a_arr = np.zeros((128, T, NJ), f32)
        for r in range(RPC):
            gidx, A = mats[slots[k][r]]
            tchunks = len(gidx) // 128
            for t in range(tchunks):
                col = int(col0[r]) + t
                logical[col * 128:(col + 1) * 128] = gidx[t * 128:(t + 1) * 128]
                a_arr[:, col, :] = A[t * 128:(t + 1) * 128, :]
        # wrap-16 + replicate to 128 partitions (see _build_program)
        idx16 = np.tile(logical.astype(np.int16).reshape(-1, 16).T, (8, 1))
        in_maps.append({"fcl": fcl, "amat": a_arr, "pidx": idx16})

    res = run_bass_kernel_spmd(nc, in_maps, core_ids=list(range(N_CORES)))

    out_full = np.empty((N_ROIS, C, P, P), f32)
    for k in range(N_CORES):
        arr = res.results[k]["out"]  # (128, RPC, 2, 49)
        t = arr.transpose(1, 2, 0, 3).reshape(RPC, C, P, P)
        out_full[slots[k]] = t
    return out_full


# revision 8
# speedup vs baseline: 1.8301x; 1.4927x over previous
"""DCNv2 deformable PS-RoI pooling on 8 Trainium2 NeuronCores.

Strategy (RoI-data-parallel, 32 rois per core):
  * Host replicates the reference coordinate math exactly in float32 and folds
    bilinear weights, validity masking and the 1/count normalization into a
    per-roi sparse matrix A (bbox_pixels x 49). Each roi touches only a small
    bbox of the 64x64 feature map, so A has ~128-384 rows (padded to 128k).
  * Feature map is transposed to channel-last (B*H*W, C) on host so each pixel
    is a contiguous 1KB channel vector in HBM.
  * Device (SPMD, identical program on 8 cores, per-core data in DRAM inputs):
      - one indirect-DMA gather per column group: patch[p, t, :] =
        Fcl[idx[p, t], :]  -> pixel-on-partition layout
      - per roi: out(c,j) accumulated in PSUM over 128-pixel chunks via
        matmul(lhsT=patch_chunk(128px, 128c), rhs=A_chunk(128px, 49j))
      - PSUM -> SBUF staging -> one contiguous DMA to HBM (c-major scratch
        layout); host undoes the layout permutation while assembling.
"""
import numpy as np

f32 = np.float32
f64 = np.float64

B, C, H, W = 8, 256, 64, 64
N_ROIS, P, S = 256, 7, 4
PART = 7
NJ = P * P  # 49
SCALE = f32(1.0 / 16.0)
TRANS_STD = f32(0.1)
N_CORES = 8
RPC = N_ROIS // N_CORES  # rois per core
N_GROUPS = 6  # gather/compute pipeline groups

_prog_cache = {}


# --------------------------------------------------------------------------
# host math: exact f32 replication of the reference coordinate computation
# --------------------------------------------------------------------------
def _roi_sampling_data(rois, offset):
    rois = np.asarray(rois, dtype=f32)
    offset = np.asarray(offset, dtype=f32)
    batch = rois[:, 0].astype(np.int32)

    roi_sw = np.round(rois[:, 1]) * SCALE - f32(0.5)
    roi_sh = np.round(rois[:, 2]) * SCALE - f32(0.5)
    roi_ew = (np.round(rois[:, 3]) + f32(1.0)) * SCALE - f32(0.5)
    roi_eh = (np.round(rois[:, 4]) + f32(1.0)) * SCALE - f32(0.5)
    roi_w = np.maximum(roi_ew - roi_sw, f32(0.1))
    roi_h = np.maximum(roi_eh - roi_sh, f32(0.1))
    bin_w = roi_w / f32(P)
    bin_h = roi_h / f32(P)
    sub_w = bin_w / f32(S)
    sub_h = bin_h / f32(S)

    ph = np.arange(P, dtype=np.int32)
    pw = np.arange(P, dtype=np.int32)
    part_h = np.clip(
        np.floor(ph.astype(f32) / f32(P) * f32(PART)).astype(np.int32), 0, PART - 1
    )
    part_w = np.clip(
        np.floor(pw.astype(f32) / f32(P) * f32(PART)).astype(np.int32), 0, PART - 1
    )

    tx = offset[:, 0][:, part_h[:, None], part_w[None, :]] * TRANS_STD  # (N,7,7)
    ty = offset[:, 1][:, part_h[:, None], part_w[None, :]] * TRANS_STD

    wstart = (
        pw.astype(f32)[None, None, :] * bin_w[:, None, None]
        + roi_sw[:, None, None]
        + tx * roi_w[:, None, None]
    )
    hstart = (
        ph.astype(f32)[None, :, None] * bin_h[:, None, None]
        + roi_sh[:, None, None]
        + ty * roi_h[:, None, None]
    )

    iw = np.arange(S, dtype=f32)
    ih = np.arange(S, dtype=f32)
    wpos = (
        wstart[:, :, :, None, None]
        + iw[None, None, None, None, :] * sub_w[:, None, None, None, None]
    )
    hpos = (
        hstart[:, :, :, None, None]
        + ih[None, None, None, :, None] * sub_h[:, None, None, None, None]
    )

    valid = (
        (wpos >= f32(-0.5)) & (wpos <= f32(W) - f32(0.5))
        & (hpos >= f32(-0.5)) & (hpos <= f32(H) - f32(0.5))
    )
    wc = np.clip(wpos, f32(0.0), f32(W - 1.0))
    hc = np.clip(hpos, f32(0.0), f32(H - 1.0))

    x0 = np.floor(wc).astype(np.int32)
    x1 = np.ceil(wc).astype(np.int32)
    y0 = np.floor(hc).astype(np.int32)
    y1 = np.ceil(hc).astype(np.int32)
    dx = (wc - np.floor(wc)).astype(f64)
    dy = (hc - np.floor(hc)).astype(f64)

    cnt = valid.sum(axis=(3, 4)).astype(f32)  # (N,7,7)
    coef = np.where(cnt > 0, 1.0 / np.maximum(cnt, f32(1.0)).astype(f64), 0.0)

    w00 = (1.0 - dx) * (1.0 - dy)
    w01 = dx * (1.0 - dy)
    w10 = (1.0 - dx) * dy
    w11 = dx * dy

    return dict(
        batch=batch, valid=valid, x0=x0, x1=x1, y0=y0, y1=y1,
        w00=w00, w01=w01, w10=w10, w11=w11, coef=coef,
    )


def _build_roi_mats(rois, offset):
    """Per roi: (pixel idx int32 (npix,), A f32 (npix, 49)), npix % 128 == 0."""
    d = _roi_sampling_data(rois, offset)
    j_grid = np.arange(NJ, dtype=np.int64).reshape(P, P, 1, 1)
    j_grid = np.broadcast_to(j_grid, (P, P, S, S))
    full = (P, P, S, S)

    out = []
    for n in range(N_ROIS):
        v = d["valid"][n]
        if not v.any():
            out.append((np.zeros(128, np.int32), np.zeros((128, NJ), f32)))
            continue
        jj = j_grid[v]
        xs0 = np.broadcast_to(d["x0"][n], full)[v]
        xs1 = np.broadcast_to(d["x1"][n], full)[v]
        ys0 = np.broadcast_to(d["y0"][n], full)[v]
        ys1 = np.broadcast_to(d["y1"][n], full)[v]
        cf = np.broadcast_to(d["coef"][n][:, :, None, None], full)[v]
        bx0 = int(xs0.min()); bx1 = int(xs1.max())
        by0 = int(ys0.min()); by1 = int(ys1.max())
        bw = bx1 - bx0 + 1
        bh = by1 - by0 + 1
        npix = bh * bw
        npad = (-npix) % 128
        A = np.zeros((npix + npad, NJ), f64)
        for yy, xx, ww in (
            (ys0, xs0, np.broadcast_to(d["w00"][n], full)[v]),
            (ys0, xs1, np.broadcast_to(d["w01"][n], full)[v]),
            (ys1, xs0, np.broadcast_to(d["w10"][n], full)[v]),
            (ys1, xs1, np.broadcast_to(d["w11"][n], full)[v]),
        ):
            lp = (yy - by0).astype(np.int64) * bw + (xx - bx0)
            np.add.at(A, (lp, jj), ww * cf)
        yidx = (by0 + np.arange(bh, dtype=np.int32))[:, None]
        xidx = (bx0 + np.arange(bw, dtype=np.int32))[None, :]
        gidx = (int(d["batch"][n]) * (H * W) + yidx * W + xidx).reshape(-1)
        gidx = np.concatenate([gidx, np.zeros(npad, np.int32)]).astype(np.int32)
        out.append((gidx, A.astype(f32)))
    return out


# --------------------------------------------------------------------------
# device program
# --------------------------------------------------------------------------
def _build_program(nch):
    """nch: tuple of RPC ints = chunks per roi slot. Same program on 8 cores."""
    import concourse.bacc as bacc
    import concourse.bass as bass
    import concourse.mybir as mybir
    from concourse.tile import TileContext

    T = int(sum(nch))
    col0 = np.concatenate([[0], np.cumsum(nch)]).astype(int)  # slot -> first col

    # split slots into N_GROUPS groups with roughly equal column counts
    bounds = [0]
    for g in range(1, N_GROUPS):
        target = T * g / N_GROUPS
        s = int(np.searchsorted(col0, target))
        s = min(max(s, bounds[-1] + 1), RPC - (N_GROUPS - g))
        bounds.append(s)
    bounds.append(RPC)

    nc = bacc.Bacc("TRN2", num_devices=N_CORES)
    dt = mybir.dt
    fcl = nc.dram_tensor("fcl", [B * H * W, C], dt.float16, kind="ExternalInput")
    amat = nc.dram_tensor("amat", [128, T, NJ], dt.float16, kind="ExternalInput")
    # dma_gather index layout: logical idx i lives at [i % 16, i // 16],
    # replicated across the 8 groups of 16 partitions.
    pidx = nc.dram_tensor("pidx", [128, T * 8], dt.int16, kind="ExternalInput")
    outd = nc.dram_tensor("out", [128, RPC, 2, NJ], dt.float16, kind="ExternalOutput")

    with TileContext(nc) as tc:
        with (
            tc.tile_pool(name="main", bufs=1) as mp,
            tc.tile_pool(name="psum", bufs=2, space="PSUM") as pp,
        ):
            idx_t = mp.tile([128, T * 8], dt.int16, tag="idx")
            nc.sync.dma_start(out=idx_t[:], in_=pidx[:])

            for g in range(N_GROUPS):
                s0, s1 = bounds[g], bounds[g + 1]
                c0, c1 = int(col0[s0]), int(col0[s1])
                ncols = c1 - c0
                a_t = mp.tile([128, ncols, NJ], dt.float16, tag=f"amat{g}")
                nc.sync.dma_start(out=a_t[:], in_=amat[:, c0:c1, :])
                p_t = mp.tile([128, ncols, C], dt.float16, tag=f"patch{g}")
                nc.gpsimd.dma_gather(
                    out_ap=p_t[:],
                    in_ap=fcl[:],
                    idxs_ap=idx_t[:, c0 * 8:c1 * 8],
                    num_idxs=ncols * 128,
                    num_idxs_reg=ncols * 128,
                    elem_size=C,
                    single_packet=False,
                )
                ob = mp.tile([128, s1 - s0, 2, NJ], dt.float16, tag=f"outbuf{g}")
                # pack 5 rois (10 roi-halves x 49) per PSUM bank; one DVE
                # copy per bank instead of one per roi-half, then one
                # output DMA per bank so the tail drains early
                for b0 in range(s0, s1, 5):
                    b1 = min(b0 + 5, s1)
                    nsl = (b1 - b0) * 2
                    pb = pp.tile([128, nsl * NJ], dt.float32, tag="pbank")
                    for r in range(b0, b1):
                        for h in range(2):
                            o = ((r - b0) * 2 + h) * NJ
                            for t in range(nch[r]):
                                c = int(col0[r]) - c0 + t
                                nc.tensor.matmul(
                                    out=pb[:, o:o + NJ],
                                    lhsT=p_t[:, c, h * 128:(h + 1) * 128],
                                    rhs=a_t[:, c, :],
                                    start=(t == 0),
                                    stop=(t == nch[r] - 1),
                                )
                    nc.vector.tensor_copy(
                        out=ob[:, b0 - s0:b1 - s0, :, :], in_=pb[:, :nsl * NJ]
                    )
                    nc.sync.dma_start(
                        out=outd[:, b0:b1, :, :], in_=ob[:, b0 - s0:b1 - s0, :, :]
                    )
    nc.compile()
    return nc


# --------------------------------------------------------------------------
# entry point
# --------------------------------------------------------------------------
def _partition_rois(mats):
    """Snake-deal rois to cores by descending chunk count so every slot r
    holds 8 near-equal-size rois -> per-slot max (nch) is tight."""
    chunks_per = np.array([len(g) // 128 for g, _ in mats])
    order = np.argsort(-chunks_per, kind="stable")
    slots = [[None] * RPC for _ in range(N_CORES)]  # slots[k][r] = roi index
    for i, roi in enumerate(order):
        rnd, pos = divmod(i, N_CORES)
        core = pos if rnd % 2 == 0 else N_CORES - 1 - pos
        slots[core][rnd] = int(roi)
    slots = [np.array(s) for s in slots]
    nch = tuple(
        int(max(chunks_per[slots[k][r]] for k in range(N_CORES))) for r in range(RPC)
    )
    return slots, nch


def kernel(input, rois, offset):
    from concourse.bass_utils import run_bass_kernel_spmd

    input = np.asarray(input, dtype=f32)
    mats = _build_roi_mats(rois, offset)

    fcl = np.ascontiguousarray(
        input.transpose(0, 2, 3, 1).astype(np.float16)
    ).reshape(B * H * W, C)

    slots, nch = _partition_rois(mats)
    T = int(sum(nch))
    col0 = np.concatenate([[0], np.cumsum(nch)]).astype(int)

    key = nch
    if key not in _prog_cache:
        _prog_cache[key] = _build_program(nch)
    nc = _prog_cache[key]

    in_maps = []
    for k in range(N_CORES):
        logical = np.zeros(T * 128, np.int32)
        a_arr = np.zeros((128, T, NJ), np.float16)
        for r in range(RPC):
            gidx, A = mats[slots[k][r]]
            tchunks = len(gidx) // 128
            for t in range(tchunks):
                col = int(col0[r]) + t
                logical[col * 128:(col + 1) * 128] = gidx[t * 128:(t + 1) * 128]
                a_arr[:, col, :] = A[t * 128:(t + 1) * 128, :]
        # wrap-16 + replicate to 128 partitions (see _build_program)
        idx16 = np.tile(logical.astype(np.int16).reshape(-1, 16).T, (8, 1))
        in_maps.append({"fcl": fcl, "amat": a_arr, "pidx": idx16})

    res = run_bass_kernel_spmd(nc, in_maps, core_ids=list(range(N_CORES)))

    out_full = np.empty((N_ROIS, C, P, P), f32)
    for k in range(N_CORES):
        arr = res.results[k]["out"].astype(f32)  # (128, RPC, 2, 49)
        t = arr.transpose(1, 2, 0, 3).reshape(RPC, C, P, P)
        out_full[slots[k]] = t
    return out_full


# revision 16
# speedup vs baseline: 1.8761x; 1.0251x over previous
"""DCNv2 deformable PS-RoI pooling on 8 Trainium2 NeuronCores.

Strategy (RoI-data-parallel, 32 rois per core):
  * Host replicates the reference coordinate math exactly in float32 and folds
    bilinear weights, validity masking and the 1/count normalization into a
    per-roi sparse matrix A (bbox_pixels x 49). Each roi touches only a small
    bbox of the 64x64 feature map, so A has ~128-384 rows (padded to 128k).
  * Feature map is transposed to channel-last (B*H*W, C) on host so each pixel
    is a contiguous 1KB channel vector in HBM.
  * Device (SPMD, identical program on 8 cores, per-core data in DRAM inputs):
      - one indirect-DMA gather per column group: patch[p, t, :] =
        Fcl[idx[p, t], :]  -> pixel-on-partition layout
      - per roi: out(c,j) accumulated in PSUM over 128-pixel chunks via
        matmul(lhsT=patch_chunk(128px, 128c), rhs=A_chunk(128px, 49j))
      - PSUM -> SBUF staging -> one contiguous DMA to HBM (c-major scratch
        layout); host undoes the layout permutation while assembling.
"""
import numpy as np

f32 = np.float32
f64 = np.float64

B, C, H, W = 8, 256, 64, 64
N_ROIS, P, S = 256, 7, 4
PART = 7
NJ = P * P  # 49
SCALE = f32(1.0 / 16.0)
TRANS_STD = f32(0.1)
N_CORES = 8
RPC = N_ROIS // N_CORES  # rois per core
N_GROUPS = 6  # gather/compute pipeline groups
GROUP_WEIGHTS = [0.5, 1.4, 1.4, 1.2, 1.0, 0.5, 0.4, 0.3]  # truncated to N_GROUPS
A_SPLIT = False  # upload A per group instead of one shot

_prog_cache = {}


# --------------------------------------------------------------------------
# host math: exact f32 replication of the reference coordinate computation
# --------------------------------------------------------------------------
def _roi_sampling_data(rois, offset):
    rois = np.asarray(rois, dtype=f32)
    offset = np.asarray(offset, dtype=f32)
    batch = rois[:, 0].astype(np.int32)

    roi_sw = np.round(rois[:, 1]) * SCALE - f32(0.5)
    roi_sh = np.round(rois[:, 2]) * SCALE - f32(0.5)
    roi_ew = (np.round(rois[:, 3]) + f32(1.0)) * SCALE - f32(0.5)
    roi_eh = (np.round(rois[:, 4]) + f32(1.0)) * SCALE - f32(0.5)
    roi_w = np.maximum(roi_ew - roi_sw, f32(0.1))
    roi_h = np.maximum(roi_eh - roi_sh, f32(0.1))
    bin_w = roi_w / f32(P)
    bin_h = roi_h / f32(P)
    sub_w = bin_w / f32(S)
    sub_h = bin_h / f32(S)

    ph = np.arange(P, dtype=np.int32)
    pw = np.arange(P, dtype=np.int32)
    part_h = np.clip(
        np.floor(ph.astype(f32) / f32(P) * f32(PART)).astype(np.int32), 0, PART - 1
    )
    part_w = np.clip(
        np.floor(pw.astype(f32) / f32(P) * f32(PART)).astype(np.int32), 0, PART - 1
    )

    tx = offset[:, 0][:, part_h[:, None], part_w[None, :]] * TRANS_STD  # (N,7,7)
    ty = offset[:, 1][:, part_h[:, None], part_w[None, :]] * TRANS_STD

    wstart = (
        pw.astype(f32)[None, None, :] * bin_w[:, None, None]
        + roi_sw[:, None, None]
        + tx * roi_w[:, None, None]
    )
    hstart = (
        ph.astype(f32)[None, :, None] * bin_h[:, None, None]
        + roi_sh[:, None, None]
        + ty * roi_h[:, None, None]
    )

    iw = np.arange(S, dtype=f32)
    ih = np.arange(S, dtype=f32)
    wpos = (
        wstart[:, :, :, None, None]
        + iw[None, None, None, None, :] * sub_w[:, None, None, None, None]
    )
    hpos = (
        hstart[:, :, :, None, None]
        + ih[None, None, None, :, None] * sub_h[:, None, None, None, None]
    )

    valid = (
        (wpos >= f32(-0.5)) & (wpos <= f32(W) - f32(0.5))
        & (hpos >= f32(-0.5)) & (hpos <= f32(H) - f32(0.5))
    )
    wc = np.clip(wpos, f32(0.0), f32(W - 1.0))
    hc = np.clip(hpos, f32(0.0), f32(H - 1.0))

    x0 = np.floor(wc).astype(np.int32)
    x1 = np.ceil(wc).astype(np.int32)
    y0 = np.floor(hc).astype(np.int32)
    y1 = np.ceil(hc).astype(np.int32)
    dx = (wc - np.floor(wc)).astype(f64)
    dy = (hc - np.floor(hc)).astype(f64)

    cnt = valid.sum(axis=(3, 4)).astype(f32)  # (N,7,7)
    coef = np.where(cnt > 0, 1.0 / np.maximum(cnt, f32(1.0)).astype(f64), 0.0)

    w00 = (1.0 - dx) * (1.0 - dy)
    w01 = dx * (1.0 - dy)
    w10 = (1.0 - dx) * dy
    w11 = dx * dy

    return dict(
        batch=batch, valid=valid, x0=x0, x1=x1, y0=y0, y1=y1,
        w00=w00, w01=w01, w10=w10, w11=w11, coef=coef,
    )


def _build_roi_mats(rois, offset):
    """Per roi: (pixel idx int32 (npix,), A f32 (npix, 49)), npix % 128 == 0."""
    d = _roi_sampling_data(rois, offset)
    j_grid = np.arange(NJ, dtype=np.int64).reshape(P, P, 1, 1)
    j_grid = np.broadcast_to(j_grid, (P, P, S, S))
    full = (P, P, S, S)

    out = []
    for n in range(N_ROIS):
        v = d["valid"][n]
        if not v.any():
            out.append((np.zeros(128, np.int32), np.zeros((128, NJ), f32)))
            continue
        jj = j_grid[v]
        xs0 = np.broadcast_to(d["x0"][n], full)[v]
        xs1 = np.broadcast_to(d["x1"][n], full)[v]
        ys0 = np.broadcast_to(d["y0"][n], full)[v]
        ys1 = np.broadcast_to(d["y1"][n], full)[v]
        cf = np.broadcast_to(d["coef"][n][:, :, None, None], full)[v]
        bx0 = int(xs0.min()); bx1 = int(xs1.max())
        by0 = int(ys0.min()); by1 = int(ys1.max())
        bw = bx1 - bx0 + 1
        bh = by1 - by0 + 1
        npix = bh * bw
        npad = (-npix) % 128
        A = np.zeros((npix + npad, NJ), f64)
        for yy, xx, ww in (
            (ys0, xs0, np.broadcast_to(d["w00"][n], full)[v]),
            (ys0, xs1, np.broadcast_to(d["w01"][n], full)[v]),
            (ys1, xs0, np.broadcast_to(d["w10"][n], full)[v]),
            (ys1, xs1, np.broadcast_to(d["w11"][n], full)[v]),
        ):
            lp = (yy - by0).astype(np.int64) * bw + (xx - bx0)
            np.add.at(A, (lp, jj), ww * cf)
        yidx = (by0 + np.arange(bh, dtype=np.int32))[:, None]
        xidx = (bx0 + np.arange(bw, dtype=np.int32))[None, :]
        gidx = (int(d["batch"][n]) * (H * W) + yidx * W + xidx).reshape(-1)
        gidx = np.concatenate([gidx, np.zeros(npad, np.int32)]).astype(np.int32)
        out.append((gidx, A.astype(f32)))
    return out


# --------------------------------------------------------------------------
# device program
# --------------------------------------------------------------------------
def _build_program(nch):
    """nch: tuple of RPC ints = chunks per roi slot. Same program on 8 cores."""
    import concourse.bacc as bacc
    import concourse.bass as bass
    import concourse.mybir as mybir
    from concourse.tile import TileContext

    T = int(sum(nch))
    col0 = np.concatenate([[0], np.cumsum(nch)]).astype(int)  # slot -> first col

    # split slots into N_GROUPS groups; group 0 small so the pipeline
    # starts early, last groups small so the tail drains fast
    weights = GROUP_WEIGHTS[:N_GROUPS]
    cum = np.cumsum(weights) / sum(weights)
    bounds = [0]
    for g in range(N_GROUPS - 1):
        target = T * cum[g]
        s = int(np.searchsorted(col0, target))
        s = min(max(s, bounds[-1] + 1), RPC - (N_GROUPS - 1 - g))
        bounds.append(s)
    bounds.append(RPC)

    nc = bacc.Bacc("TRN2", num_devices=N_CORES)
    dt = mybir.dt
    fcl = nc.dram_tensor("fcl", [B * H * W, C], dt.float16, kind="ExternalInput")
    amat = nc.dram_tensor("amat", [128, T, NJ], dt.float16, kind="ExternalInput")
    # dma_gather index layout: logical idx i lives at [i % 16, i // 16],
    # replicated across the 8 groups of 16 partitions.
    pidx = nc.dram_tensor("pidx", [128, T * 8], dt.int16, kind="ExternalInput")
    outd = nc.dram_tensor("out", [128, RPC, 2, NJ], dt.float16, kind="ExternalOutput")

    with TileContext(nc) as tc:
        with (
            tc.tile_pool(name="main", bufs=1) as mp,
            tc.tile_pool(name="psum", bufs=2, space="PSUM") as pp,
        ):
            idx_t = mp.tile([128, T * 8], dt.int16, tag="idx")
            nc.sync.dma_start(out=idx_t[:], in_=pidx[:])
            if not A_SPLIT:
                a_full = mp.tile([128, T, NJ], dt.float16, tag="amat")
                nc.sync.dma_start(out=a_full[:], in_=amat[:])

            for g in range(N_GROUPS):
                s0, s1 = bounds[g], bounds[g + 1]
                c0, c1 = int(col0[s0]), int(col0[s1])
                ncols = c1 - c0
                if A_SPLIT:
                    a_g = mp.tile([128, ncols, NJ], dt.float16, tag=f"amat{g}")
                    nc.scalar.dma_start(out=a_g[:], in_=amat[:, c0:c1, :])
                p_t = mp.tile([128, ncols, C], dt.float16, tag=f"patch{g}")
                nc.gpsimd.dma_gather(
                    out_ap=p_t[:],
                    in_ap=fcl[:],
                    idxs_ap=idx_t[:, c0 * 8:c1 * 8],
                    num_idxs=ncols * 128,
                    num_idxs_reg=ncols * 128,
                    elem_size=C,
                    single_packet=False,
                )
                ob = mp.tile([128, s1 - s0, 2, NJ], dt.float16, tag=f"outbuf{g}")
                # pack 5 rois (10 roi-halves x 49) per PSUM bank; one DVE
                # copy per bank instead of one per roi-half
                for b0 in range(s0, s1, 5):
                    b1 = min(b0 + 5, s1)
                    nsl = (b1 - b0) * 2
                    pb = pp.tile([128, nsl * NJ], dt.float32, tag="pbank")
                    for r in range(b0, b1):
                        for h in range(2):
                            o = ((r - b0) * 2 + h) * NJ
                            for t in range(nch[r]):
                                c = int(col0[r]) + t
                                rhs = (
                                    a_g[:, c - c0, :] if A_SPLIT
                                    else a_full[:, c, :]
                                )
                                nc.tensor.matmul(
                                    out=pb[:, o:o + NJ],
                                    lhsT=p_t[:, c - c0, h * 128:(h + 1) * 128],
                                    rhs=rhs,
                                    start=(t == 0),
                                    stop=(t == nch[r] - 1),
                                )
                    nc.vector.tensor_copy(
                        out=ob[:, b0 - s0:b1 - s0, :, :], in_=pb[:, :nsl * NJ]
                    )
                # one output DMA per group; the last group drains per-bank
                # via the loop above having filled ob fully
                nc.sync.dma_start(out=outd[:, s0:s1, :, :], in_=ob[:])
    nc.compile()
    return nc


# --------------------------------------------------------------------------
# entry point
# --------------------------------------------------------------------------
def _partition_rois(mats):
    """Snake-deal rois to cores by descending chunk count so every slot r
    holds 8 near-equal-size rois -> per-slot max (nch) is tight."""
    chunks_per = np.array([len(g) // 128 for g, _ in mats])
    order = np.argsort(-chunks_per, kind="stable")
    slots = [[None] * RPC for _ in range(N_CORES)]  # slots[k][r] = roi index
    for i, roi in enumerate(order):
        rnd, pos = divmod(i, N_CORES)
        core = pos if rnd % 2 == 0 else N_CORES - 1 - pos
        slots[core][rnd] = int(roi)
    slots = [np.array(s) for s in slots]
    nch = tuple(
        int(max(chunks_per[slots[k][r]] for k in range(N_CORES))) for r in range(RPC)
    )
    return slots, nch


def kernel(input, rois, offset):
    from concourse.bass_utils import run_bass_kernel_spmd

    input = np.asarray(input, dtype=f32)
    mats = _build_roi_mats(rois, offset)

    fcl = np.ascontiguousarray(
        input.transpose(0, 2, 3, 1).astype(np.float16)
    ).reshape(B * H * W, C)

    slots, nch = _partition_rois(mats)
    T = int(sum(nch))
    col0 = np.concatenate([[0], np.cumsum(nch)]).astype(int)

    key = nch
    if key not in _prog_cache:
        _prog_cache[key] = _build_program(nch)
    nc = _prog_cache[key]

    in_maps = []
    for k in range(N_CORES):
        logical = np.zeros(T * 128, np.int32)
        a_arr = np.zeros((128, T, NJ), np.float16)
        for r in range(RPC):
            gidx, A = mats[slots[k][r]]
            tchunks = len(gidx) // 128
            for t in range(tchunks):
                col = int(col0[r]) + t
                logical[col * 128:(col + 1) * 128] = gidx[t * 128:(t + 1) * 128]
                a_arr[:, col, :] = A[t * 128:(t + 1) * 128, :]
        # wrap-16 + replicate to 128 partitions (see _build_program)
        idx16 = np.tile(logical.astype(np.int16).reshape(-1, 16).T, (8, 1))
        in_maps.append({"fcl": fcl, "amat": a_arr, "pidx": idx16})

    res = run_bass_kernel_spmd(nc, in_maps, core_ids=list(range(N_CORES)))

    out_full = np.empty((N_ROIS, C, P, P), f32)
    for k in range(N_CORES):
        arr = res.results[k]["out"].astype(f32)  # (128, RPC, 2, 49)
        t = arr.transpose(1, 2, 0, 3).reshape(RPC, C, P, P)
        out_full[slots[k]] = t
    return out_full
